# revision 3
# baseline (speedup 1.0000x reference)
"""Relational GNN layer  y = sum_r A_r @ X @ W_r^T  on 8 trn2 NeuronCores.

Sharding: relation-parallel. Core c handles relation c:
    Y_c = A_c @ (X @ W_c^T)          (A_c: [N, N], X: [N, F], W_c: [F, F])
Host sums the 8 partial [N, F] outputs.

Device layout trick: the tensor engine contracts along the partition dim of
both operands, and A's contraction index is its minor dim.  So the host
passes A_c^T (contiguous), X^T and W_c^T, and the kernel computes
    Z = X @ W_c^T          via  out[j,f] = sum_k xt[k,j] * wt[k,f]
    Y_c^T = Z^T @ A_c^T    via  out[f,i] = sum_j  Z[j,f] * at[j,i]
with every SBUF tile loaded in its natural (row-major) layout.
Output is returned as Y_c^T [F, N]; host sums and transposes.

Shapes are hardcoded for R=8, N=4096, F_IN=F_OUT=128, fp32.
"""

import numpy as np

R, N, F = 8, 4096, 128
JBLK = N // 128          # 32 contraction chunks of 128
NCORES = 8
HALF = N // 2            # i-range covered per PSUM pass
QPH = HALF // 512        # 512-wide matmuls per pass (4)

_CACHE = {}


def _build_program():
    import concourse.mybir as mybir
    import concourse.tile as tile
    from concourse import bacc

    dt = mybir.dt
    nc = bacc.Bacc("TRN2", target_bir_lowering=False, debug=False)

    at = nc.dram_tensor("at", [N, N], dt.float32, kind="ExternalInput").ap()
    xt = nc.dram_tensor("xt", [F, N], dt.float32, kind="ExternalInput").ap()
    wt = nc.dram_tensor("wt", [F, F], dt.float32, kind="ExternalInput").ap()
    yt = nc.dram_tensor("yt", [F, N], dt.float32, kind="ExternalOutput").ap()

    with tile.TileContext(nc) as tc:
        with (
            tc.sbuf_pool(name="const", bufs=1) as cpool,
            tc.psum_pool(name="zp", bufs=2) as zp,
            tc.sbuf_pool(name="astripes", bufs=4) as apool,
            tc.psum_pool(name="yp", bufs=4) as yp,
        ):
            xt_s = cpool.tile([128, N], dt.float32)
            nc.sync.dma_start(out=xt_s[:], in_=xt)
            wt_s = cpool.tile([128, F], dt.float32)
            nc.sync.dma_start(out=wt_s[:], in_=wt)

            # z_all[:, jb*128+f] = Z[jb*128+p, f] = (X @ W_c^T)[jb*128+p, f]
            z_all = cpool.tile([128, N], dt.float32)
            for jb in range(JBLK):
                zps = zp.tile([128, F], dt.float32, tag="zps")
                nc.tensor.matmul(
                    zps[:],
                    lhsT=xt_s[:, jb * 128 : (jb + 1) * 128],
                    rhs=wt_s[:],
                    start=True,
                    stop=True,
                )
                nc.vector.tensor_copy(z_all[:, jb * 128 : (jb + 1) * 128], zps[:])

            yt_sb = cpool.tile([128, N], dt.float32)
            for half in range(2):
                accs = [
                    yp.tile([128, 512], dt.float32, tag="yacc", name=f"yacc{half}_{q}")
                    for q in range(QPH)
                ]
                for jc in range(JBLK):
                    astr = apool.tile([128, HALF], dt.float32, tag="astr", name=f"astr{half}_{jc}")
                    nc.sync.dma_start(
                        out=astr[:],
                        in_=at[jc * 128 : (jc + 1) * 128, half * HALF : (half + 1) * HALF],
                    )
                    for q in range(QPH):
                        nc.tensor.matmul(
                            accs[q][:],
                            lhsT=z_all[:, jc * 128 : (jc + 1) * 128],
                            rhs=astr[:, q * 512 : (q + 1) * 512],
                            start=(jc == 0),
                            stop=(jc == JBLK - 1),
                        )
                for q in range(QPH):
                    nc.vector.tensor_copy(
                        yt_sb[:, half * HALF + q * 512 : half * HALF + (q + 1) * 512],
                        accs[q][:],
                    )
            nc.sync.dma_start(out=yt, in_=yt_sb[:])

    nc.compile()
    return nc


def _ensure_ntff_hook():
    """The image's antenv lacks axon_hooks; synthesize it so bass_utils'
    trace=True path can capture NTFF profiles via the axon .so."""
    import sys
    import types

    try:
        from antenv.axon_hooks import get_axon_ntff_profile_hook  # noqa: F401

        return
    except ImportError:
        pass

    mod = types.ModuleType("antenv.axon_hooks")
    _hook = [None]
    mod.set_axon_ntff_profile_hook = lambda h: _hook.__setitem__(0, h)
    mod.get_axon_ntff_profile_hook = lambda: _hook[0]
    sys.modules["antenv.axon_hooks"] = mod
    import antenv

    antenv.axon_hooks = mod
    try:
        from trn_agent_boot.trn_boot import _ntff_profile_via_ctypes

        mod.set_axon_ntff_profile_hook(
            _ntff_profile_via_ctypes("/opt/axon/libaxon_pjrt.so")
        )
    except Exception:
        pass

    # Keep artifact handling local — no share/S3 in this container.
    import concourse.bass_utils as bu

    bu.upload_artifacts = lambda tmpdir: tmpdir


def kernel(adjacency, features, weight, _trace=False, _tmpdir=None):
    from concourse.bass_utils import run_bass_kernel_spmd

    if _trace:
        _ensure_ntff_hook()

    if "nc" not in _CACHE:
        _CACHE["nc"] = _build_program()
    nc = _CACHE["nc"]

    adjacency = np.ascontiguousarray(adjacency, dtype=np.float32)
    xt_np = np.ascontiguousarray(features.T, dtype=np.float32)
    in_maps = [
        {
            "at": np.ascontiguousarray(adjacency[c].T),
            "xt": xt_np,
            "wt": np.ascontiguousarray(weight[c].T, dtype=np.float32),
        }
        for c in range(NCORES)
    ]

    res = run_bass_kernel_spmd(
        nc, in_maps, core_ids=list(range(NCORES)), trace=_trace, tmpdir=_tmpdir
    )
    _CACHE["last_exec_ns"] = res.exec_time_ns
    _CACHE["last_results"] = res

    yt_sum = np.zeros((F, N), dtype=np.float32)
    for r in res.results:
        yt_sum += r["yt"]
    return np.ascontiguousarray(yt_sum.T)


# revision 7
# speedup vs baseline: 1.3221x; 1.3221x over previous
"""Relational GNN layer  y = sum_r A_r @ X @ W_r^T  on 8 trn2 NeuronCores.

Sharding: relation-parallel. Core c handles relation c:
    Y_c = A_c @ (X @ W_c^T)          (A_c: [N, N], X: [N, F], W_c: [F, F])
Host sums the 8 partial [N, F] outputs.

Device layout trick: the tensor engine contracts along the partition dim of
both operands, and A's contraction index is its minor dim.  So the host
passes A_c^T (contiguous), X^T and W_c^T, and the kernel computes
    Z = X @ W_c^T          via  out[j,f] = sum_k xt[k,j] * wt[k,f]
    Y_c^T = Z^T @ A_c^T    via  out[f,i] = sum_j  Z[j,f] * at[j,i]
with every SBUF tile loaded in its natural (row-major) layout.
Output is returned as Y_c^T [F, N]; host sums and transposes.

Shapes are hardcoded for R=8, N=4096, F_IN=F_OUT=128, fp32.
"""

import numpy as np

R, N, F = 8, 4096, 128
JBLK = N // 128          # 32 contraction chunks of 128
NCORES = 8
HALF = N // 2            # i-range covered per PSUM pass
QPH = HALF // 512        # 512-wide matmuls per pass (4)

_CACHE = {}


def _build_program():
    import concourse.mybir as mybir
    import concourse.tile as tile
    from concourse import bacc

    dt = mybir.dt
    nc = bacc.Bacc("TRN2", target_bir_lowering=False, debug=False)

    at = nc.dram_tensor("at", [N, N], dt.float32, kind="ExternalInput").ap()
    xt = nc.dram_tensor("xt", [F, N], dt.float32, kind="ExternalInput").ap()
    wt = nc.dram_tensor("wt", [F, F], dt.float32, kind="ExternalInput").ap()
    yt = nc.dram_tensor("yt", [F, N], dt.float32, kind="ExternalOutput").ap()

    with tile.TileContext(nc) as tc:
        with (
            tc.sbuf_pool(name="const", bufs=1) as cpool,
            tc.psum_pool(name="zp", bufs=2) as zp,
            tc.sbuf_pool(name="astripes", bufs=6) as apool,
            tc.psum_pool(name="yp", bufs=4) as yp,
        ):
            xt_s = cpool.tile([128, N], dt.float32)
            nc.sync.dma_start(out=xt_s[:], in_=xt)
            wt_s = cpool.tile([128, F], dt.float32)
            nc.sync.dma_start(out=wt_s[:], in_=wt)

            # z_all[:, jb*128+f] = Z[jb*128+p, f] = (X @ W_c^T)[jb*128+p, f]
            z_all = cpool.tile([128, N], dt.float32r)
            for jb in range(JBLK):
                zps = zp.tile([128, F], dt.float32, tag="zps")
                nc.tensor.matmul(
                    zps[:],
                    lhsT=xt_s[:, jb * 128 : (jb + 1) * 128],
                    rhs=wt_s[:],
                    start=True,
                    stop=True,
                )
                nc.vector.tensor_copy(z_all[:, jb * 128 : (jb + 1) * 128], zps[:])

            yt_sb = cpool.tile([128, N], dt.float32)
            for half in range(2):
                accs = [
                    yp.tile([128, 512], dt.float32, tag="yacc", name=f"yacc{half}_{q}")
                    for q in range(QPH)
                ]
                for jc in range(JBLK):
                    astr = apool.tile([128, HALF], dt.float32r, tag="astr", name=f"astr{half}_{jc}")
                    nc.sync.dma_start(
                        out=astr[:],
                        in_=at[
                            jc * 128 : (jc + 1) * 128, half * HALF : (half + 1) * HALF
                        ].bitcast(dt.float32r),
                    )
                    for q in range(QPH):
                        nc.tensor.matmul(
                            accs[q][:],
                            lhsT=z_all[:, jc * 128 : (jc + 1) * 128],
                            rhs=astr[:, q * 512 : (q + 1) * 512],
                            start=(jc == 0),
                            stop=(jc == JBLK - 1),
                        )
                for q in range(QPH):
                    nc.vector.tensor_copy(
                        yt_sb[:, half * HALF + q * 512 : half * HALF + (q + 1) * 512],
                        accs[q][:],
                    )
            nc.sync.dma_start(out=yt, in_=yt_sb[:])

    nc.compile()
    return nc


def _ensure_ntff_hook():
    """The image's antenv lacks axon_hooks; synthesize it so bass_utils'
    trace=True path can capture NTFF profiles via the axon .so."""
    import sys
    import types

    try:
        from antenv.axon_hooks import get_axon_ntff_profile_hook  # noqa: F401

        return
    except ImportError:
        pass

    mod = types.ModuleType("antenv.axon_hooks")
    _hook = [None]
    mod.set_axon_ntff_profile_hook = lambda h: _hook.__setitem__(0, h)
    mod.get_axon_ntff_profile_hook = lambda: _hook[0]
    sys.modules["antenv.axon_hooks"] = mod
    import antenv

    antenv.axon_hooks = mod
    try:
        from trn_agent_boot.trn_boot import _ntff_profile_via_ctypes

        mod.set_axon_ntff_profile_hook(
            _ntff_profile_via_ctypes("/opt/axon/libaxon_pjrt.so")
        )
    except Exception:
        pass

    # Keep artifact handling local — no share/S3 in this container.
    import concourse.bass_utils as bu

    bu.upload_artifacts = lambda tmpdir: tmpdir


def kernel(adjacency, features, weight, _trace=False, _tmpdir=None):
    from concourse.bass_utils import run_bass_kernel_spmd

    if _trace:
        _ensure_ntff_hook()

    if "nc" not in _CACHE:
        _CACHE["nc"] = _build_program()
    nc = _CACHE["nc"]

    adjacency = np.ascontiguousarray(adjacency, dtype=np.float32)
    xt_np = np.ascontiguousarray(features.T, dtype=np.float32)
    in_maps = [
        {
            "at": np.ascontiguousarray(adjacency[c].T),
            "xt": xt_np,
            "wt": np.ascontiguousarray(weight[c].T, dtype=np.float32),
        }
        for c in range(NCORES)
    ]

    res = run_bass_kernel_spmd(
        nc, in_maps, core_ids=list(range(NCORES)), trace=_trace, tmpdir=_tmpdir
    )
    _CACHE["last_exec_ns"] = res.exec_time_ns
    _CACHE["last_results"] = res

    yt_sum = np.zeros((F, N), dtype=np.float32)
    for r in res.results:
        yt_sum += r["yt"]
    return np.ascontiguousarray(yt_sum.T)
